# revision 20
# baseline (speedup 1.0000x reference)
"""Trainium2 Bass kernel for CausalMHA (rotary + sliding-window causal attention).

Problem: nn_CausalMHA_55319178773118
  B=4, L=2048, D=1024, H=16, Dh=64, rot_dim=32, window=512, fp32 IO.

Sharding (8 cores): data-parallel over B (4) x tensor-parallel over heads (2
groups of 8). Wqkv column-sharded, out_proj row-sharded; each core computes a
partial (full-shape) output and the host sums the 2 partials per batch.

Self-contained: hardcodes all shapes; no reads of /root/problem/*.
"""

import numpy as np
import ml_dtypes

import concourse.tile as tile
from concourse import bacc, mybir
from concourse.bass_utils import run_bass_kernel_spmd

B, L, D = 4, 2048, 1024
H, DH = 16, 64
ROT, HROT = 32, 16
WINDOW = 512
SCALE = DH ** -0.5
HPC = H // 2          # heads per core (8)
DPC = HPC * DH        # per-core head-dim width (512)
TB = 512              # token-block width for the QKV projection
NTB = L // TB         # 4
NQB = L // 128        # 16 query blocks
NEG = -30000.0        # mask additive value (exact in bf16)

f32 = mybir.dt.float32
f32r = mybir.dt.float32r
bf16 = mybir.dt.bfloat16

_CACHE = {}


def _build_module():
    """Build (once) the per-core Bass module. SPMD: same NEFF on all 8 cores."""
    if "nc" in _CACHE:
        return _CACHE["nc"]

    nc = bacc.Bacc("TRN2", target_bir_lowering=False, debug=False, num_devices=8)

    xT = nc.dram_tensor("xT", [D, L], f32r, kind="ExternalInput")
    wq = nc.dram_tensor("wq", [D, DPC], f32r, kind="ExternalInput")
    wk = nc.dram_tensor("wk", [D, DPC], f32r, kind="ExternalInput")
    wv = nc.dram_tensor("wv", [D, DPC], f32r, kind="ExternalInput")
    wo = nc.dram_tensor("wo", [DPC, D], bf16, kind="ExternalInput")
    ropec = nc.dram_tensor("ropec", [128, L], bf16, kind="ExternalInput")
    ropes = nc.dram_tensor("ropes", [128, L], bf16, kind="ExternalInput")
    mt = nc.dram_tensor("mt", [128, 128], f32r, kind="ExternalInput")
    ident = nc.dram_tensor("ident", [128, 128], bf16, kind="ExternalInput")
    mask0 = nc.dram_tensor("mask0", [128, 128], bf16, kind="ExternalInput")
    mask4 = nc.dram_tensor("mask4", [128, 128], bf16, kind="ExternalInput")
    out = nc.dram_tensor("out", [L, D], f32, kind="ExternalOutput")

    EXP = mybir.ActivationFunctionType.Exp

    with tile.TileContext(nc) as tc:
        with (
            tc.tile_pool(name="const", bufs=1) as cp,
            tc.tile_pool(name="store", bufs=1) as sp,
            tc.tile_pool(name="xt", bufs=2) as xp,
        ):
            # ---- persistent SBUF; DMA issue order tuned for startup ----
            wq_t = [cp.tile([128, DPC], f32r, tag=f"wq{d}", name=f"wq{d}") for d in range(8)]
            wk_t = [cp.tile([128, DPC], f32r, tag=f"wk{d}", name=f"wk{d}") for d in range(8)]
            wv_t = [cp.tile([128, DPC], f32r, tag=f"wv{d}", name=f"wv{d}") for d in range(8)]
            wo_t = [cp.tile([128, D], bf16, tag=f"wo{p}", name=f"wo{p}") for p in range(4)]
            rc_t = cp.tile([128, L], bf16, tag="rc", name="rc")
            rs_t = cp.tile([128, L], bf16, tag="rs", name="rs")
            mt_t = cp.tile([128, 128], f32r, tag="mt", name="mt")
            id_t = cp.tile([128, 128], bf16, tag="id", name="id")
            m0_t = cp.tile([128, 128], bf16, tag="m0", name="m0")
            m4_t = cp.tile([128, 128], bf16, tag="m4", name="m4")

            # first compute needs wq + xt(tb=0): issue those first
            xt0 = [xp.tile([128, TB], f32r, tag=f"x{d}", name=f"x{d}") for d in range(8)]
            for d in range(8):
                nc.sync.dma_start(wq_t[d][:], wq[d * 128:(d + 1) * 128, :])
                nc.sync.dma_start(xt0[d][:], xT[d * 128:(d + 1) * 128, 0:TB])
            for d in range(8):
                nc.sync.dma_start(wk_t[d][:], wk[d * 128:(d + 1) * 128, :])
            nc.sync.dma_start(mt_t[:], mt[:])
            nc.sync.dma_start(rc_t[:], ropec[:])
            nc.sync.dma_start(rs_t[:], ropes[:])
            for d in range(8):
                nc.sync.dma_start(wv_t[d][:], wv[d * 128:(d + 1) * 128, :])
            nc.sync.dma_start(id_t[:], ident[:])
            nc.sync.dma_start(m0_t[:], mask0[:])
            nc.sync.dma_start(m4_t[:], mask4[:])
            for p in range(4):
                nc.sync.dma_start(wo_t[p][:], wo[p * 128:(p + 1) * 128, :])

            # persistent activations (bf16)
            QT = [sp.tile([128, L], bf16, tag=f"QT{p}", name=f"QT{p}") for p in range(4)]
            KT = [sp.tile([128, L], bf16, tag=f"KT{p}", name=f"KT{p}") for p in range(4)]
            V = [sp.tile([128, DPC], bf16, tag=f"V{t}", name=f"V{t}") for t in range(16)]
            CT = [sp.tile([128, L], bf16, tag=f"CT{p}", name=f"CT{p}") for p in range(4)]

            # ================= Phase A: QKV projection + RoPE =================
            with (
                tc.tile_pool(name="scrA", bufs=3) as sa,
                tc.tile_pool(name="psA", bufs=2, space="PSUM") as psA,
                tc.tile_pool(name="psPr", bufs=2, space="PSUM") as psPr,
                tc.tile_pool(name="psV", bufs=2, space="PSUM") as psV,
            ):
                for tb in range(NTB):
                    ts = slice(tb * TB, (tb + 1) * TB)
                    if tb == 0:
                        xt = xt0
                    else:
                        xt = [xp.tile([128, TB], f32r, tag=f"x{d}", name=f"x{d}") for d in range(8)]
                        for d in range(8):
                            nc.sync.dma_start(xt[d][:], xT[d * 128:(d + 1) * 128, ts])
                    # QT / KT (transposed layout) + rotary
                    for hp in range(4):
                        for w_t, OUT in ((wq_t, QT), (wk_t, KT)):
                            pp = psA.tile([128, TB], f32, tag="pp", name="pp")
                            for d in range(8):
                                nc.tensor.matmul(
                                    pp[:],
                                    w_t[d][:, hp * 128:(hp + 1) * 128],
                                    xt[d][:],
                                    start=(d == 0), stop=(d == 7),
                                )
                            qs = sa.tile([128, TB], f32r, tag="qs", name="qs")
                            nc.scalar.copy(qs[:], pp[:])
                            prp = psPr.tile([128, TB], f32, tag="prp", name="prp")
                            nc.tensor.matmul(prp[:], mt_t[:], qs[:])
                            t1 = sa.tile([128, TB], f32, tag="t1", name="t1")
                            nc.vector.tensor_mul(t1[:], qs[:].bitcast(f32), rc_t[:, ts])
                            t2 = sa.tile([128, TB], f32, tag="t2", name="t2")
                            nc.vector.tensor_mul(t2[:], prp[:], rs_t[:, ts])
                            nc.vector.tensor_add(OUT[hp][:, ts], t1[:], t2[:])
                    # V (natural layout): psum [128 tok, 512 dv]
                    for sub in range(TB // 128):
                        vp = psV.tile([128, DPC], f32, tag="vp", name="vp")
                        for d in range(8):
                            nc.tensor.matmul(
                                vp[:],
                                xt[d][:, sub * 128:(sub + 1) * 128],
                                wv_t[d][:],
                                start=(d == 0), stop=(d == 7),
                            )
                        nc.scalar.copy(V[tb * (TB // 128) + sub][:], vp[:])

            # ========== Phase B+C: attention (+ fused out_proj per qb) ========
            with (
                tc.tile_pool(name="pS", bufs=2, space="PSUM") as psS,
                tc.tile_pool(name="pT", bufs=2, space="PSUM") as psT,
                tc.tile_pool(name="pC", bufs=1, space="PSUM") as psC,
                tc.tile_pool(name="pO", bufs=1, space="PSUM") as psO,
                tc.tile_pool(name="pb", bufs=4) as pb,
                tc.tile_pool(name="stat", bufs=4) as st,
                tc.tile_pool(name="pts", bufs=4) as ptp,
                tc.tile_pool(name="scrO", bufs=2) as so,
            ):
                for qb in range(NQB):
                    nb = min(qb + 1, 5)
                    kb0 = qb + 1 - nb
                    W = nb * 128
                    qsl = slice(qb * 128, (qb + 1) * 128)
                    for hp in range(4):
                        cx = psC.tile([128, 128], f32, tag="cx", name="cx")
                        HEADS = ((0, 0), (1, 64))
                        scs = {}
                        # both heads' masks+scores emitted adjacent: the two
                        # heads use PE row-groups 0:64 / 64:128 and run
                        # concurrently on hardware (tile_position row packing)
                        for hi, r0 in HEADS:
                            scs[hi] = psS.tile([128, W], f32, tag="sc", name="sc")
                            nc.tensor.matmul(
                                scs[hi][:, (nb - 1) * 128:W], id_t[:], m0_t[:],
                                start=True, stop=False, skip_group_check=True,
                            )
                            if nb == 5:
                                nc.tensor.matmul(
                                    scs[hi][:, 0:128], id_t[:], m4_t[:],
                                    start=True, stop=False, skip_group_check=True,
                                )
                        for hi, r0 in HEADS:
                            rsl = slice(r0, r0 + 64)
                            if nb == 5:
                                nc.tensor.matmul(
                                    scs[hi][:, 0:512],
                                    QT[hp][rsl, qsl],
                                    KT[hp][rsl, kb0 * 128:kb0 * 128 + 512],
                                    start=False, stop=True, skip_group_check=True,
                                )
                            else:
                                nc.tensor.matmul(
                                    scs[hi][:, 0:W],
                                    QT[hp][rsl, qsl],
                                    KT[hp][rsl, kb0 * 128:(qb + 1) * 128],
                                    start=False, stop=True, skip_group_check=True,
                                )
                        if nb == 5:
                            for hi, r0 in HEADS:
                                rsl = slice(r0, r0 + 64)
                                nc.tensor.matmul(
                                    scs[hi][:, 512:640],
                                    QT[hp][rsl, qsl],
                                    KT[hp][rsl, qb * 128:(qb + 1) * 128],
                                    start=False, stop=True, skip_group_check=True,
                                )
                        # exp (unnormalized) + per-query row sums
                        pp_, tps, ptss = {}, {}, {}
                        for hi, r0 in HEADS:
                            h = 2 * hp + hi
                            pp_[hi] = pb.tile([128, W], bf16, tag="p", name="p")
                            ssum = st.tile([128, 1], f32, tag="ss", name="ss")
                            nc.scalar.activation(
                                pp_[hi][:], scs[hi][:], EXP, scale=SCALE,
                                accum_out=ssum[:],
                            )
                            rr = st.tile([128, 1], f32, tag="rr", name="rr")
                            nc.vector.reciprocal(rr[:], ssum[:])
                            nc.vector.tensor_scalar_mul(pp_[hi][:], pp_[hi][:], rr[:])
                        # transpose P blocks (batched psum tile) + DVE copy out
                        for hi, r0 in HEADS:
                            tps[hi] = psT.tile([128, W], bf16, tag="tp", name="tp")
                            for j in range(nb):
                                nc.tensor.transpose(
                                    tps[hi][:, j * 128:(j + 1) * 128],
                                    pp_[hi][:, j * 128:(j + 1) * 128],
                                    id_t[:],
                                )
                            ptss[hi] = ptp.tile([128, W], bf16, tag="pt", name="pt")
                            nc.vector.tensor_copy(ptss[hi][:], tps[hi][:])
                        # PV interleaved across heads: col-groups 0:64 / 64:128
                        # of the PE array run concurrently on hardware
                        for j in range(nb):
                            kb = kb0 + j
                            for hi, r0 in HEADS:
                                h = 2 * hp + hi
                                nc.tensor.matmul(
                                    cx[r0:r0 + 64, :],
                                    V[kb][:, h * 64:h * 64 + 64],
                                    ptss[hi][:, j * 128:(j + 1) * 128],
                                    start=(j == 0), stop=(j == nb - 1),
                                    tile_position=(0, r0),
                                    skip_group_check=True,
                                )
                        nc.vector.tensor_copy(CT[hp][:, qsl], cx[:])
                    # fused out_proj for token block qb
                    for eb in range(2):
                        op = psO.tile([128, 512], f32, tag="op", name="op")
                        for hp in range(4):
                            nc.tensor.matmul(
                                op[:],
                                CT[hp][:, qsl],
                                wo_t[hp][:, eb * 512:(eb + 1) * 512],
                                start=(hp == 0), stop=(hp == 3),
                            )
                        ob = so.tile([128, 512], f32, tag="ob", name="ob")
                        nc.vector.tensor_copy(ob[:], op[:])
                        nc.sync.dma_start(out[qsl, eb * 512:(eb + 1) * 512], ob[:])

    nc.finalize()
    _CACHE["nc"] = nc
    return nc


def _host_tables():
    if "tabs" in _CACHE:
        return _CACHE["tabs"]
    inv = 1.0 / (10000.0 ** (np.arange(0, ROT, 2, dtype=np.float32) / ROT))
    t = np.arange(L, dtype=np.float32)
    fr = np.outer(t, inv)                      # [L, 16]
    cos, sin = np.cos(fr), np.sin(fr)          # [L, 16]
    rc = np.ones((128, L), dtype=np.float32)
    rs = np.zeros((128, L), dtype=np.float32)
    for base in (0, 64):
        rc[base:base + HROT] = cos.T
        rc[base + HROT:base + ROT] = cos.T
        rs[base:base + HROT] = sin.T
        rs[base + HROT:base + ROT] = sin.T
    # shift matrix M: partner = M @ q  (per 64-dim head block)
    M = np.zeros((128, 128), dtype=np.float32)
    for base in (0, 64):
        for j in range(HROT):
            M[base + j, base + j + HROT] = -1.0
            M[base + j + HROT, base + j] = 1.0
    mt = np.ascontiguousarray(M.T)
    ident = np.eye(128, dtype=np.float32).astype(ml_dtypes.bfloat16)
    pq = np.arange(128)[:, None]
    fk = np.arange(128)[None, :]
    mask0 = np.where(fk <= pq, 0.0, NEG).astype(ml_dtypes.bfloat16)
    mask4 = np.where(fk > pq, 0.0, NEG).astype(ml_dtypes.bfloat16)
    _CACHE["tabs"] = (
        rc.astype(ml_dtypes.bfloat16), rs.astype(ml_dtypes.bfloat16),
        mt, ident, mask0, mask4,
    )
    return _CACHE["tabs"]


def kernel(x, attn_mask, Wqkv_kernel, out_proj_kernel):
    x = np.asarray(x, dtype=np.float32)
    Wqkv = np.asarray(Wqkv_kernel, dtype=np.float32)
    Wo = np.asarray(out_proj_kernel, dtype=np.float32)
    rc, rs, mt, ident, mask0, mask4 = _host_tables()
    nc = _build_module()

    in_maps = []
    for c in range(8):
        b, hg = c // 2, c % 2
        sl = slice(hg * DPC, (hg + 1) * DPC)
        in_maps.append({
            "xT": np.ascontiguousarray(x[b].T),
            "wq": np.ascontiguousarray(Wqkv[:, hg * DPC:(hg + 1) * DPC]),
            "wk": np.ascontiguousarray(Wqkv[:, D + hg * DPC:D + (hg + 1) * DPC]),
            "wv": np.ascontiguousarray(Wqkv[:, 2 * D + hg * DPC:2 * D + (hg + 1) * DPC]),
            "wo": np.ascontiguousarray(Wo[sl, :]).astype(ml_dtypes.bfloat16),
            "ropec": rc, "ropes": rs, "mt": mt,
            "ident": ident, "mask0": mask0, "mask4": mask4,
        })

    res = run_bass_kernel_spmd(nc, in_maps, core_ids=list(range(8)))
    _CACHE["last_res"] = res
    outs = [r["out"] for r in res.results]
    full = np.stack([outs[2 * b] + outs[2 * b + 1] for b in range(B)])
    return full.astype(np.float32)


# revision 21
# speedup vs baseline: 1.0069x; 1.0069x over previous
"""Trainium2 Bass kernel for CausalMHA (rotary + sliding-window causal attention).

Problem: nn_CausalMHA_55319178773118
  B=4, L=2048, D=1024, H=16, Dh=64, rot_dim=32, window=512, fp32 IO.

Sharding (8 cores): data-parallel over B (4) x tensor-parallel over heads (2
groups of 8). Wqkv column-sharded, out_proj row-sharded; each core computes a
partial (full-shape) output and the host sums the 2 partials per batch.

Self-contained: hardcodes all shapes; no reads of /root/problem/*.
"""

import numpy as np
import ml_dtypes

import concourse.tile as tile
from concourse import bacc, mybir
from concourse.bass_utils import run_bass_kernel_spmd

B, L, D = 4, 2048, 1024
H, DH = 16, 64
ROT, HROT = 32, 16
WINDOW = 512
SCALE = DH ** -0.5
HPC = H // 2          # heads per core (8)
DPC = HPC * DH        # per-core head-dim width (512)
TB = 512              # token-block width for the QKV projection
NTB = L // TB         # 4
NQB = L // 128        # 16 query blocks
NEG = -30000.0        # mask additive value (exact in bf16)

f32 = mybir.dt.float32
f32r = mybir.dt.float32r
bf16 = mybir.dt.bfloat16

_CACHE = {}


def _build_module():
    """Build (once) the per-core Bass module. SPMD: same NEFF on all 8 cores."""
    if "nc" in _CACHE:
        return _CACHE["nc"]

    nc = bacc.Bacc("TRN2", target_bir_lowering=False, debug=False, num_devices=8)

    xT = nc.dram_tensor("xT", [D, L], f32r, kind="ExternalInput")
    wq = nc.dram_tensor("wq", [D, DPC], f32r, kind="ExternalInput")
    wk = nc.dram_tensor("wk", [D, DPC], f32r, kind="ExternalInput")
    wv = nc.dram_tensor("wv", [D, DPC], f32r, kind="ExternalInput")
    wo = nc.dram_tensor("wo", [DPC, D], bf16, kind="ExternalInput")
    ropec = nc.dram_tensor("ropec", [128, L], bf16, kind="ExternalInput")
    ropes = nc.dram_tensor("ropes", [128, L], bf16, kind="ExternalInput")
    mt = nc.dram_tensor("mt", [128, 128], f32r, kind="ExternalInput")
    ident = nc.dram_tensor("ident", [128, 128], bf16, kind="ExternalInput")
    mask0 = nc.dram_tensor("mask0", [128, 128], bf16, kind="ExternalInput")
    mask4 = nc.dram_tensor("mask4", [128, 128], bf16, kind="ExternalInput")
    out = nc.dram_tensor("out", [L, D], f32, kind="ExternalOutput")

    EXP = mybir.ActivationFunctionType.Exp

    with tile.TileContext(nc) as tc:
        with (
            tc.tile_pool(name="const", bufs=1) as cp,
            tc.tile_pool(name="store", bufs=1) as sp,
            tc.tile_pool(name="xt", bufs=2) as xp,
        ):
            # ---- persistent SBUF; DMA issue order tuned for startup ----
            wq_t = [cp.tile([128, DPC], f32r, tag=f"wq{d}", name=f"wq{d}") for d in range(8)]
            wk_t = [cp.tile([128, DPC], f32r, tag=f"wk{d}", name=f"wk{d}") for d in range(8)]
            wv_t = [cp.tile([128, DPC], f32r, tag=f"wv{d}", name=f"wv{d}") for d in range(8)]
            wo_t = [cp.tile([128, D], bf16, tag=f"wo{p}", name=f"wo{p}") for p in range(4)]
            rc_t = cp.tile([128, L], bf16, tag="rc", name="rc")
            rs_t = cp.tile([128, L], bf16, tag="rs", name="rs")
            mt_t = cp.tile([128, 128], f32r, tag="mt", name="mt")
            id_t = cp.tile([128, 128], bf16, tag="id", name="id")
            m0_t = cp.tile([128, 128], bf16, tag="m0", name="m0")
            m4_t = cp.tile([128, 128], bf16, tag="m4", name="m4")

            # first compute needs wq + xt(tb=0): issue those first
            xt0 = [xp.tile([128, TB], f32r, tag=f"x{d}", name=f"x{d}") for d in range(8)]
            for d in range(8):
                nc.sync.dma_start(wq_t[d][:], wq[d * 128:(d + 1) * 128, :])
                nc.sync.dma_start(xt0[d][:], xT[d * 128:(d + 1) * 128, 0:TB])
            for d in range(8):
                nc.sync.dma_start(wk_t[d][:], wk[d * 128:(d + 1) * 128, :])
            nc.sync.dma_start(mt_t[:], mt[:])
            nc.sync.dma_start(rc_t[:], ropec[:])
            nc.sync.dma_start(rs_t[:], ropes[:])
            for d in range(8):
                nc.sync.dma_start(wv_t[d][:], wv[d * 128:(d + 1) * 128, :])
            nc.sync.dma_start(id_t[:], ident[:])
            nc.sync.dma_start(m0_t[:], mask0[:])
            nc.sync.dma_start(m4_t[:], mask4[:])
            for p in range(4):
                nc.sync.dma_start(wo_t[p][:], wo[p * 128:(p + 1) * 128, :])

            # persistent activations (bf16)
            QT = [sp.tile([128, L], bf16, tag=f"QT{p}", name=f"QT{p}") for p in range(4)]
            KT = [sp.tile([128, L], bf16, tag=f"KT{p}", name=f"KT{p}") for p in range(4)]
            V = [sp.tile([128, DPC], bf16, tag=f"V{t}", name=f"V{t}") for t in range(16)]
            CT = [sp.tile([128, L], bf16, tag=f"CT{p}", name=f"CT{p}") for p in range(4)]

            # ================= Phase A: QKV projection + RoPE =================
            with (
                tc.tile_pool(name="scrA", bufs=3) as sa,
                tc.tile_pool(name="psA", bufs=3, space="PSUM") as psA,
                tc.tile_pool(name="psPr", bufs=2, space="PSUM") as psPr,
                tc.tile_pool(name="psV", bufs=3, space="PSUM") as psV,
            ):
                for tb in range(NTB):
                    ts = slice(tb * TB, (tb + 1) * TB)
                    if tb == 0:
                        xt = xt0
                    else:
                        xt = [xp.tile([128, TB], f32r, tag=f"x{d}", name=f"x{d}") for d in range(8)]
                        for d in range(8):
                            nc.sync.dma_start(xt[d][:], xT[d * 128:(d + 1) * 128, ts])
                    # QT / KT (transposed layout) + rotary
                    for hp in range(4):
                        for w_t, OUT in ((wq_t, QT), (wk_t, KT)):
                            pp = psA.tile([128, TB], f32, tag="pp", name="pp")
                            for d in range(8):
                                nc.tensor.matmul(
                                    pp[:],
                                    w_t[d][:, hp * 128:(hp + 1) * 128],
                                    xt[d][:],
                                    start=(d == 0), stop=(d == 7),
                                )
                            qs = sa.tile([128, TB], f32r, tag="qs", name="qs")
                            nc.scalar.copy(qs[:], pp[:])
                            prp = psPr.tile([128, TB], f32, tag="prp", name="prp")
                            nc.tensor.matmul(prp[:], mt_t[:], qs[:])
                            t1 = sa.tile([128, TB], f32, tag="t1", name="t1")
                            nc.vector.tensor_mul(t1[:], qs[:].bitcast(f32), rc_t[:, ts])
                            t2 = sa.tile([128, TB], f32, tag="t2", name="t2")
                            nc.vector.tensor_mul(t2[:], prp[:], rs_t[:, ts])
                            nc.vector.tensor_add(OUT[hp][:, ts], t1[:], t2[:])
                    # V (natural layout): psum [128 tok, 512 dv]
                    for sub in range(TB // 128):
                        vp = psV.tile([128, DPC], f32, tag="vp", name="vp")
                        for d in range(8):
                            nc.tensor.matmul(
                                vp[:],
                                xt[d][:, sub * 128:(sub + 1) * 128],
                                wv_t[d][:],
                                start=(d == 0), stop=(d == 7),
                            )
                        nc.scalar.copy(V[tb * (TB // 128) + sub][:], vp[:])

            # ========== Phase B+C: attention (+ fused out_proj per qb) ========
            with (
                tc.tile_pool(name="pS", bufs=2, space="PSUM") as psS,
                tc.tile_pool(name="pT", bufs=2, space="PSUM") as psT,
                tc.tile_pool(name="pC", bufs=1, space="PSUM") as psC,
                tc.tile_pool(name="pO", bufs=1, space="PSUM") as psO,
                tc.tile_pool(name="pb", bufs=4) as pb,
                tc.tile_pool(name="stat", bufs=4) as st,
                tc.tile_pool(name="pts", bufs=4) as ptp,
                tc.tile_pool(name="scrO", bufs=2) as so,
            ):
                for qb in range(NQB):
                    nb = min(qb + 1, 5)
                    kb0 = qb + 1 - nb
                    W = nb * 128
                    qsl = slice(qb * 128, (qb + 1) * 128)
                    for hp in range(4):
                        cx = psC.tile([128, 128], f32, tag="cx", name="cx")
                        HEADS = ((0, 0), (1, 64))
                        scs = {}
                        # both heads' masks+scores emitted adjacent: the two
                        # heads use PE row-groups 0:64 / 64:128 and run
                        # concurrently on hardware (tile_position row packing)
                        for hi, r0 in HEADS:
                            scs[hi] = psS.tile([128, W], f32, tag="sc", name="sc")
                            nc.tensor.matmul(
                                scs[hi][:, (nb - 1) * 128:W], id_t[:], m0_t[:],
                                start=True, stop=False, skip_group_check=True,
                            )
                            if nb == 5:
                                nc.tensor.matmul(
                                    scs[hi][:, 0:128], id_t[:], m4_t[:],
                                    start=True, stop=False, skip_group_check=True,
                                )
                        for hi, r0 in HEADS:
                            rsl = slice(r0, r0 + 64)
                            if nb == 5:
                                nc.tensor.matmul(
                                    scs[hi][:, 0:512],
                                    QT[hp][rsl, qsl],
                                    KT[hp][rsl, kb0 * 128:kb0 * 128 + 512],
                                    start=False, stop=True, skip_group_check=True,
                                )
                            else:
                                nc.tensor.matmul(
                                    scs[hi][:, 0:W],
                                    QT[hp][rsl, qsl],
                                    KT[hp][rsl, kb0 * 128:(qb + 1) * 128],
                                    start=False, stop=True, skip_group_check=True,
                                )
                        if nb == 5:
                            for hi, r0 in HEADS:
                                rsl = slice(r0, r0 + 64)
                                nc.tensor.matmul(
                                    scs[hi][:, 512:640],
                                    QT[hp][rsl, qsl],
                                    KT[hp][rsl, qb * 128:(qb + 1) * 128],
                                    start=False, stop=True, skip_group_check=True,
                                )
                        # exp (unnormalized) + per-query row sums
                        pp_, tps, ptss = {}, {}, {}
                        for hi, r0 in HEADS:
                            h = 2 * hp + hi
                            pp_[hi] = pb.tile([128, W], bf16, tag="p", name="p")
                            ssum = st.tile([128, 1], f32, tag="ss", name="ss")
                            nc.scalar.activation(
                                pp_[hi][:], scs[hi][:], EXP, scale=SCALE,
                                accum_out=ssum[:],
                            )
                            rr = st.tile([128, 1], f32, tag="rr", name="rr")
                            nc.vector.reciprocal(rr[:], ssum[:])
                            nc.vector.tensor_scalar_mul(pp_[hi][:], pp_[hi][:], rr[:])
                        # transpose P blocks (batched psum tile) + DVE copy out
                        for hi, r0 in HEADS:
                            tps[hi] = psT.tile([128, W], bf16, tag="tp", name="tp")
                            for j in range(nb):
                                nc.tensor.transpose(
                                    tps[hi][:, j * 128:(j + 1) * 128],
                                    pp_[hi][:, j * 128:(j + 1) * 128],
                                    id_t[:],
                                )
                            ptss[hi] = ptp.tile([128, W], bf16, tag="pt", name="pt")
                            nc.vector.tensor_copy(ptss[hi][:], tps[hi][:])
                        # PV interleaved across heads: col-groups 0:64 / 64:128
                        # of the PE array run concurrently on hardware
                        for j in range(nb):
                            kb = kb0 + j
                            for hi, r0 in HEADS:
                                h = 2 * hp + hi
                                nc.tensor.matmul(
                                    cx[r0:r0 + 64, :],
                                    V[kb][:, h * 64:h * 64 + 64],
                                    ptss[hi][:, j * 128:(j + 1) * 128],
                                    start=(j == 0), stop=(j == nb - 1),
                                    tile_position=(0, r0),
                                    skip_group_check=True,
                                )
                        nc.vector.tensor_copy(CT[hp][:, qsl], cx[:])
                    # fused out_proj for token block qb
                    for eb in range(2):
                        op = psO.tile([128, 512], f32, tag="op", name="op")
                        for hp in range(4):
                            nc.tensor.matmul(
                                op[:],
                                CT[hp][:, qsl],
                                wo_t[hp][:, eb * 512:(eb + 1) * 512],
                                start=(hp == 0), stop=(hp == 3),
                            )
                        ob = so.tile([128, 512], f32, tag="ob", name="ob")
                        nc.vector.tensor_copy(ob[:], op[:])
                        nc.sync.dma_start(out[qsl, eb * 512:(eb + 1) * 512], ob[:])

    nc.finalize()
    _CACHE["nc"] = nc
    return nc


def _host_tables():
    if "tabs" in _CACHE:
        return _CACHE["tabs"]
    inv = 1.0 / (10000.0 ** (np.arange(0, ROT, 2, dtype=np.float32) / ROT))
    t = np.arange(L, dtype=np.float32)
    fr = np.outer(t, inv)                      # [L, 16]
    cos, sin = np.cos(fr), np.sin(fr)          # [L, 16]
    rc = np.ones((128, L), dtype=np.float32)
    rs = np.zeros((128, L), dtype=np.float32)
    for base in (0, 64):
        rc[base:base + HROT] = cos.T
        rc[base + HROT:base + ROT] = cos.T
        rs[base:base + HROT] = sin.T
        rs[base + HROT:base + ROT] = sin.T
    # shift matrix M: partner = M @ q  (per 64-dim head block)
    M = np.zeros((128, 128), dtype=np.float32)
    for base in (0, 64):
        for j in range(HROT):
            M[base + j, base + j + HROT] = -1.0
            M[base + j + HROT, base + j] = 1.0
    mt = np.ascontiguousarray(M.T)
    ident = np.eye(128, dtype=np.float32).astype(ml_dtypes.bfloat16)
    pq = np.arange(128)[:, None]
    fk = np.arange(128)[None, :]
    mask0 = np.where(fk <= pq, 0.0, NEG).astype(ml_dtypes.bfloat16)
    mask4 = np.where(fk > pq, 0.0, NEG).astype(ml_dtypes.bfloat16)
    _CACHE["tabs"] = (
        rc.astype(ml_dtypes.bfloat16), rs.astype(ml_dtypes.bfloat16),
        mt, ident, mask0, mask4,
    )
    return _CACHE["tabs"]


def kernel(x, attn_mask, Wqkv_kernel, out_proj_kernel):
    x = np.asarray(x, dtype=np.float32)
    Wqkv = np.asarray(Wqkv_kernel, dtype=np.float32)
    Wo = np.asarray(out_proj_kernel, dtype=np.float32)
    rc, rs, mt, ident, mask0, mask4 = _host_tables()
    nc = _build_module()

    in_maps = []
    for c in range(8):
        b, hg = c // 2, c % 2
        sl = slice(hg * DPC, (hg + 1) * DPC)
        in_maps.append({
            "xT": np.ascontiguousarray(x[b].T),
            "wq": np.ascontiguousarray(Wqkv[:, hg * DPC:(hg + 1) * DPC]),
            "wk": np.ascontiguousarray(Wqkv[:, D + hg * DPC:D + (hg + 1) * DPC]),
            "wv": np.ascontiguousarray(Wqkv[:, 2 * D + hg * DPC:2 * D + (hg + 1) * DPC]),
            "wo": np.ascontiguousarray(Wo[sl, :]).astype(ml_dtypes.bfloat16),
            "ropec": rc, "ropes": rs, "mt": mt,
            "ident": ident, "mask0": mask0, "mask4": mask4,
        })

    res = run_bass_kernel_spmd(nc, in_maps, core_ids=list(range(8)))
    _CACHE["last_res"] = res
    outs = [r["out"] for r in res.results]
    full = np.stack([outs[2 * b] + outs[2 * b + 1] for b in range(B)])
    return full.astype(np.float32)


# revision 26
# speedup vs baseline: 1.0214x; 1.0144x over previous
"""Trainium2 Bass kernel for CausalMHA (rotary + sliding-window causal attention).

Problem: nn_CausalMHA_55319178773118
  B=4, L=2048, D=1024, H=16, Dh=64, rot_dim=32, window=512, fp32 IO.

Sharding (8 cores): data-parallel over B (4) x tensor-parallel over heads (2
groups of 8). Wqkv column-sharded, out_proj row-sharded; each core computes a
partial (full-shape) output and the host sums the 2 partials per batch.

Self-contained: hardcodes all shapes; no reads of /root/problem/*.
"""

import numpy as np
import ml_dtypes

import concourse.tile as tile
from concourse import bacc, mybir
from concourse.bass_utils import run_bass_kernel_spmd

B, L, D = 4, 2048, 1024
H, DH = 16, 64
ROT, HROT = 32, 16
WINDOW = 512
SCALE = DH ** -0.5
HPC = H // 2          # heads per core (8)
DPC = HPC * DH        # per-core head-dim width (512)
TB = 512              # token-block width for the QKV projection
NTB = L // TB         # 4
NQB = L // 128        # 16 query blocks
NEG = -30000.0        # mask additive value (exact in bf16)

f32 = mybir.dt.float32
f32r = mybir.dt.float32r
bf16 = mybir.dt.bfloat16

_CACHE = {}


def _build_module():
    """Build (once) the per-core Bass module. SPMD: same NEFF on all 8 cores."""
    if "nc" in _CACHE:
        return _CACHE["nc"]

    nc = bacc.Bacc("TRN2", target_bir_lowering=False, debug=False, num_devices=8)

    xT = nc.dram_tensor("xT", [D, L], f32r, kind="ExternalInput")
    wq = nc.dram_tensor("wq", [D, DPC], f32r, kind="ExternalInput")
    wk = nc.dram_tensor("wk", [D, DPC], f32r, kind="ExternalInput")
    wv = nc.dram_tensor("wv", [D, DPC], f32r, kind="ExternalInput")
    wo = nc.dram_tensor("wo", [DPC, D], bf16, kind="ExternalInput")
    ropec = nc.dram_tensor("ropec", [128, L], bf16, kind="ExternalInput")
    ropes = nc.dram_tensor("ropes", [128, L], bf16, kind="ExternalInput")
    mt = nc.dram_tensor("mt", [128, 128], f32r, kind="ExternalInput")
    ident = nc.dram_tensor("ident", [128, 128], bf16, kind="ExternalInput")
    mask0 = nc.dram_tensor("mask0", [128, 128], bf16, kind="ExternalInput")
    mask4 = nc.dram_tensor("mask4", [128, 128], bf16, kind="ExternalInput")
    out = nc.dram_tensor("out", [L, D], f32, kind="ExternalOutput")

    EXP = mybir.ActivationFunctionType.Exp

    with tile.TileContext(nc) as tc:
        with (
            tc.tile_pool(name="const", bufs=1) as cp,
            tc.tile_pool(name="store", bufs=1) as sp,
            tc.tile_pool(name="xt", bufs=2) as xp,
        ):
            # ---- persistent SBUF; DMA issue order tuned for startup ----
            wq_t = [cp.tile([128, DPC], f32r, tag=f"wq{d}", name=f"wq{d}") for d in range(8)]
            wk_t = [cp.tile([128, DPC], f32r, tag=f"wk{d}", name=f"wk{d}") for d in range(8)]
            wv_t = [cp.tile([128, DPC], f32r, tag=f"wv{d}", name=f"wv{d}") for d in range(8)]
            wo_t = [cp.tile([128, D], bf16, tag=f"wo{p}", name=f"wo{p}") for p in range(4)]
            rc_t = cp.tile([128, L], bf16, tag="rc", name="rc")
            rs_t = cp.tile([128, L], bf16, tag="rs", name="rs")
            mt_t = cp.tile([128, 128], f32r, tag="mt", name="mt")
            id_t = cp.tile([128, 128], bf16, tag="id", name="id")
            m0_t = cp.tile([128, 128], bf16, tag="m0", name="m0")
            m4_t = cp.tile([128, 128], bf16, tag="m4", name="m4")

            # first compute needs wq + xt(tb=0): issue those first
            xt0 = [xp.tile([128, TB], f32r, tag=f"x{d}", name=f"x{d}") for d in range(8)]
            for d in range(8):
                nc.sync.dma_start(wq_t[d][:], wq[d * 128:(d + 1) * 128, :])
                nc.sync.dma_start(xt0[d][:], xT[d * 128:(d + 1) * 128, 0:TB])
            for d in range(8):
                nc.sync.dma_start(wk_t[d][:], wk[d * 128:(d + 1) * 128, :])
            nc.sync.dma_start(mt_t[:], mt[:])
            nc.sync.dma_start(rc_t[:], ropec[:])
            nc.sync.dma_start(rs_t[:], ropes[:])
            for d in range(8):
                nc.sync.dma_start(wv_t[d][:], wv[d * 128:(d + 1) * 128, :])
            nc.sync.dma_start(id_t[:], ident[:])
            nc.sync.dma_start(m0_t[:], mask0[:])
            nc.sync.dma_start(m4_t[:], mask4[:])
            for p in range(4):
                nc.sync.dma_start(wo_t[p][:], wo[p * 128:(p + 1) * 128, :])

            # persistent activations (bf16)
            QT = [sp.tile([128, L], bf16, tag=f"QT{p}", name=f"QT{p}") for p in range(4)]
            KT = [sp.tile([128, L], bf16, tag=f"KT{p}", name=f"KT{p}") for p in range(4)]
            V = [sp.tile([128, DPC], bf16, tag=f"V{t}", name=f"V{t}") for t in range(16)]
            CT = [sp.tile([128, L], bf16, tag=f"CT{p}", name=f"CT{p}") for p in range(4)]

            # ================= Phase A: QKV projection + RoPE =================
            with (
                tc.tile_pool(name="scrA", bufs=3) as sa,
                tc.tile_pool(name="psA", bufs=3, space="PSUM") as psA,
                tc.tile_pool(name="psPr", bufs=2, space="PSUM") as psPr,
                tc.tile_pool(name="psV", bufs=3, space="PSUM") as psV,
            ):
                for tb in range(NTB):
                    ts = slice(tb * TB, (tb + 1) * TB)
                    if tb == 0:
                        xt = xt0
                    else:
                        xt = [xp.tile([128, TB], f32r, tag=f"x{d}", name=f"x{d}") for d in range(8)]
                        for d in range(8):
                            nc.sync.dma_start(xt[d][:], xT[d * 128:(d + 1) * 128, ts])
                    # QT / KT (transposed layout) + rotary
                    for hp in range(4):
                        for w_t, OUT in ((wq_t, QT), (wk_t, KT)):
                            pp = psA.tile([128, TB], f32, tag="pp", name="pp")
                            for d in range(8):
                                nc.tensor.matmul(
                                    pp[:],
                                    w_t[d][:, hp * 128:(hp + 1) * 128],
                                    xt[d][:],
                                    start=(d == 0), stop=(d == 7),
                                )
                            qs = sa.tile([128, TB], f32r, tag="qs", name="qs")
                            nc.scalar.copy(qs[:], pp[:])
                            prp = psPr.tile([128, TB], f32, tag="prp", name="prp")
                            nc.tensor.matmul(prp[:], mt_t[:], qs[:])
                            t1 = sa.tile([128, TB], f32, tag="t1", name="t1")
                            nc.vector.tensor_mul(t1[:], qs[:].bitcast(f32), rc_t[:, ts])
                            t2 = sa.tile([128, TB], f32, tag="t2", name="t2")
                            nc.vector.tensor_mul(t2[:], prp[:], rs_t[:, ts])
                            nc.vector.tensor_add(OUT[hp][:, ts], t1[:], t2[:])
                    # V (natural layout): psum [128 tok, 512 dv]
                    for sub in range(TB // 128):
                        vp = psV.tile([128, DPC], f32, tag="vp", name="vp")
                        for d in range(8):
                            nc.tensor.matmul(
                                vp[:],
                                xt[d][:, sub * 128:(sub + 1) * 128],
                                wv_t[d][:],
                                start=(d == 0), stop=(d == 7),
                            )
                        nc.scalar.copy(V[tb * (TB // 128) + sub][:], vp[:])

            # ========== Phase B+C: attention (+ fused out_proj per qb) ========
            with (
                tc.tile_pool(name="pS", bufs=2, space="PSUM") as psS,
                tc.tile_pool(name="pT", bufs=2, space="PSUM") as psT,
                tc.tile_pool(name="pC", bufs=1, space="PSUM") as psC,
                tc.tile_pool(name="pO", bufs=1, space="PSUM") as psO,
                tc.tile_pool(name="pb", bufs=4) as pb,
                tc.tile_pool(name="stat", bufs=4) as st,
                tc.tile_pool(name="pts", bufs=4) as ptp,
                tc.tile_pool(name="scrO", bufs=2) as so,
            ):
                for qb in range(NQB):
                    nb = min(qb + 1, 5)
                    kb0 = qb + 1 - nb
                    W = nb * 128
                    qsl = slice(qb * 128, (qb + 1) * 128)
                    for hp in range(4):
                        cx = psC.tile([128, 128], f32, tag="cx", name="cx")
                        HEADS = ((0, 0), (1, 64))
                        scs = {}
                        # both heads' masks+scores emitted adjacent: the two
                        # heads use PE row-groups 0:64 / 64:128 and run
                        # concurrently on hardware (tile_position row packing)
                        for hi, r0 in HEADS:
                            scs[hi] = psS.tile([128, W], f32, tag="sc", name="sc")
                            nc.tensor.matmul(
                                scs[hi][:, (nb - 1) * 128:W], id_t[:], m0_t[:],
                                start=True, stop=False, skip_group_check=True,
                            )
                            if nb == 5:
                                nc.tensor.matmul(
                                    scs[hi][:, 0:128], id_t[:], m4_t[:],
                                    start=True, stop=False, skip_group_check=True,
                                )
                        for hi, r0 in HEADS:
                            rsl = slice(r0, r0 + 64)
                            if nb == 5:
                                nc.tensor.matmul(
                                    scs[hi][:, 0:512],
                                    QT[hp][rsl, qsl],
                                    KT[hp][rsl, kb0 * 128:kb0 * 128 + 512],
                                    start=False, stop=True, skip_group_check=True,
                                )
                            else:
                                nc.tensor.matmul(
                                    scs[hi][:, 0:W],
                                    QT[hp][rsl, qsl],
                                    KT[hp][rsl, kb0 * 128:(qb + 1) * 128],
                                    start=False, stop=True, skip_group_check=True,
                                )
                        if nb == 5:
                            for hi, r0 in HEADS:
                                rsl = slice(r0, r0 + 64)
                                nc.tensor.matmul(
                                    scs[hi][:, 512:640],
                                    QT[hp][rsl, qsl],
                                    KT[hp][rsl, qb * 128:(qb + 1) * 128],
                                    start=False, stop=True, skip_group_check=True,
                                )
                        # exp (unnormalized) + per-query row sums
                        pp_, tps, ptss = {}, {}, {}
                        for hi, r0 in HEADS:
                            h = 2 * hp + hi
                            pp_[hi] = pb.tile([128, W], bf16, tag="p", name="p")
                            ssum = st.tile([128, 1], f32, tag="ss", name="ss")
                            nc.scalar.activation(
                                pp_[hi][:], scs[hi][:], EXP, scale=SCALE,
                                accum_out=ssum[:],
                            )
                            rr = st.tile([128, 1], f32, tag="rr", name="rr")
                            nc.vector.reciprocal(rr[:], ssum[:])
                            h0 = (W // 256) * 128
                            if h0 == 0:
                                nc.vector.tensor_scalar_mul(
                                    pp_[hi][:], pp_[hi][:], rr[:])
                            else:
                                nc.vector.tensor_scalar_mul(
                                    pp_[hi][:, 0:h0], pp_[hi][:, 0:h0], rr[:])
                                nc.vector.tensor_scalar_mul(
                                    pp_[hi][:, h0:W], pp_[hi][:, h0:W], rr[:])
                        # transpose P blocks (batched psum tile) + DVE copy out
                        for hi, r0 in HEADS:
                            tps[hi] = psT.tile([128, W], bf16, tag="tp", name="tp")
                            for j in range(nb):
                                nc.tensor.transpose(
                                    tps[hi][:, j * 128:(j + 1) * 128],
                                    pp_[hi][:, j * 128:(j + 1) * 128],
                                    id_t[:],
                                )
                            ptss[hi] = ptp.tile([128, W], bf16, tag="pt", name="pt")
                            nc.vector.tensor_copy(ptss[hi][:], tps[hi][:])
                        # PV interleaved across heads: col-groups 0:64 / 64:128
                        # of the PE array run concurrently on hardware
                        for j in range(nb):
                            kb = kb0 + j
                            for hi, r0 in HEADS:
                                h = 2 * hp + hi
                                nc.tensor.matmul(
                                    cx[r0:r0 + 64, :],
                                    V[kb][:, h * 64:h * 64 + 64],
                                    ptss[hi][:, j * 128:(j + 1) * 128],
                                    start=(j == 0), stop=(j == nb - 1),
                                    tile_position=(0, r0),
                                    skip_group_check=True,
                                )
                        nc.vector.tensor_copy(CT[hp][:, qsl], cx[:])
                    # fused out_proj for token block qb
                    for eb in range(2):
                        op = psO.tile([128, 512], f32, tag="op", name="op")
                        for hp in range(4):
                            nc.tensor.matmul(
                                op[:],
                                CT[hp][:, qsl],
                                wo_t[hp][:, eb * 512:(eb + 1) * 512],
                                start=(hp == 0), stop=(hp == 3),
                            )
                        ob = so.tile([128, 512], f32, tag="ob", name="ob")
                        nc.vector.tensor_copy(ob[:], op[:])
                        nc.sync.dma_start(out[qsl, eb * 512:(eb + 1) * 512], ob[:])

    nc.finalize()
    _CACHE["nc"] = nc
    return nc


def _host_tables():
    if "tabs" in _CACHE:
        return _CACHE["tabs"]
    inv = 1.0 / (10000.0 ** (np.arange(0, ROT, 2, dtype=np.float32) / ROT))
    t = np.arange(L, dtype=np.float32)
    fr = np.outer(t, inv)                      # [L, 16]
    cos, sin = np.cos(fr), np.sin(fr)          # [L, 16]
    rc = np.ones((128, L), dtype=np.float32)
    rs = np.zeros((128, L), dtype=np.float32)
    for base in (0, 64):
        rc[base:base + HROT] = cos.T
        rc[base + HROT:base + ROT] = cos.T
        rs[base:base + HROT] = sin.T
        rs[base + HROT:base + ROT] = sin.T
    # shift matrix M: partner = M @ q  (per 64-dim head block)
    M = np.zeros((128, 128), dtype=np.float32)
    for base in (0, 64):
        for j in range(HROT):
            M[base + j, base + j + HROT] = -1.0
            M[base + j + HROT, base + j] = 1.0
    mt = np.ascontiguousarray(M.T)
    ident = np.eye(128, dtype=np.float32).astype(ml_dtypes.bfloat16)
    pq = np.arange(128)[:, None]
    fk = np.arange(128)[None, :]
    mask0 = np.where(fk <= pq, 0.0, NEG).astype(ml_dtypes.bfloat16)
    mask4 = np.where(fk > pq, 0.0, NEG).astype(ml_dtypes.bfloat16)
    _CACHE["tabs"] = (
        rc.astype(ml_dtypes.bfloat16), rs.astype(ml_dtypes.bfloat16),
        mt, ident, mask0, mask4,
    )
    return _CACHE["tabs"]


def kernel(x, attn_mask, Wqkv_kernel, out_proj_kernel):
    x = np.asarray(x, dtype=np.float32)
    Wqkv = np.asarray(Wqkv_kernel, dtype=np.float32)
    Wo = np.asarray(out_proj_kernel, dtype=np.float32)
    rc, rs, mt, ident, mask0, mask4 = _host_tables()
    nc = _build_module()

    in_maps = []
    for c in range(8):
        b, hg = c // 2, c % 2
        sl = slice(hg * DPC, (hg + 1) * DPC)
        in_maps.append({
            "xT": np.ascontiguousarray(x[b].T),
            "wq": np.ascontiguousarray(Wqkv[:, hg * DPC:(hg + 1) * DPC]),
            "wk": np.ascontiguousarray(Wqkv[:, D + hg * DPC:D + (hg + 1) * DPC]),
            "wv": np.ascontiguousarray(Wqkv[:, 2 * D + hg * DPC:2 * D + (hg + 1) * DPC]),
            "wo": np.ascontiguousarray(Wo[sl, :]).astype(ml_dtypes.bfloat16),
            "ropec": rc, "ropes": rs, "mt": mt,
            "ident": ident, "mask0": mask0, "mask4": mask4,
        })

    res = run_bass_kernel_spmd(nc, in_maps, core_ids=list(range(8)))
    _CACHE["last_res"] = res
    outs = [r["out"] for r in res.results]
    full = np.stack([outs[2 * b] + outs[2 * b + 1] for b in range(B)])
    return full.astype(np.float32)
